# revision 12
# baseline (speedup 1.0000x reference)
"""2-layer GCN (PyG GCNConv, bias=False, normalize=True) on 8 TRN2 NeuronCores.

Math: out = A @ relu(A @ X @ W1) @ W2 with A = D^{-1/2} (A_w + I) D^{-1/2}.
Layer 1 is (A@X)@W1 against replicated X (zero communication); layer 2 is
A@(h1@W2) with the small per-core H2 = h1@W2 shard exchanged via 4 grouped
AllGathers that overlap layer-1 compute.

Sharding: destination nodes are block-partitioned across the 8 cores.  The
SWDGE gather ucode on GpSimd (~7ns per gathered row, serial) is the pacing
engine for layer 1, so the packing minimizes gather descriptors: edges into
each 128-row dst block are DEDUPLICATED by source node (a gathered source
row feeds all its dst columns through one multi-hot indicator column), and
self-loops plus ALL intra-core edges are folded into dense per-(dst block,
src block) [128 x 128] chunks multiplied against the core's SBUF-resident
local x rows (no descriptors at all).  Layer 2 is fully dense per
(dst block, src tile) in the permuted AllGather row space, and its TensorE
work is interleaved into the layer-1 emission (group 0 under layer 1 once
its AllGather completes; later groups immediately after) so the PE array
stays busy while GpSimd paces layer 1.
"""

import math

import numpy as np

N_CORES = 8
COMPUTE_DTYPE = "bf16"  # "f32" or "bf16"
SPLIT_BLOCKS = [3, 3, 3, 1]  # dst-block groups per collective split
CALL_CAP_TILES = 8  # idxs per call capped by the SWDGE descriptor ring


# --------------------------------------------------------------------------
# host-side graph packing
# --------------------------------------------------------------------------
def _pack_graph(edge_index, edge_weight, n_nodes, n_cores, ind_np_dtype=np.float32,
                split_blocks=None):
    src = np.asarray(edge_index[0], dtype=np.int64)
    dst = np.asarray(edge_index[1], dtype=np.int64)
    w = np.asarray(edge_weight, dtype=np.float32)

    deg = np.zeros(n_nodes, dtype=np.float32)
    np.add.at(deg, dst, w)
    deg += np.float32(1.0)
    dinv = (1.0 / np.sqrt(deg)).astype(np.float32)
    norm = (dinv[src] * w * dinv[dst]).astype(np.float32)

    npc = n_nodes // n_cores          # nodes per core
    nblk = (npc + 127) // 128         # dst blocks per core

    core = dst // npc
    dl = dst % npc
    blk = dl // 128
    col = dl % 128                    # indicator column within block
    score = src // npc
    sl = src % npc
    # edges whose src lies on the same core as their dst are handled by dense
    # per-(dst block, src block) chunks against the SBUF-resident local x rows
    # (together with the self loops), skipping the gather for them entirely
    is_local = score == core

    # ---- pass 1: per-core per-block unique-source counts (tile structure
    # must be SPMD-uniform across cores) ----
    percore = []
    uniq_counts = np.zeros((n_cores, nblk), dtype=np.int64)
    for c in range(n_cores):
        m = (core == c) & ~is_local
        sc, bc, cc_, vc = src[m], blk[m], col[m], norm[m]
        order = np.lexsort((sc, bc))
        sc, bc, cc_, vc = sc[order], bc[order], cc_[order], vc[order]
        new = np.ones(len(sc), dtype=bool)
        if len(sc) > 1:
            new[1:] = (sc[1:] != sc[:-1]) | (bc[1:] != bc[:-1])
        gid = np.cumsum(new) - 1          # unique (blk, src) group per edge
        gsrc, gblk = sc[new], bc[new]
        cntb = np.bincount(gblk, minlength=nblk)
        uniq_counts[c] = cntb
        percore.append((cc_, vc, gid, gsrc, gblk, cntb))

    t_blocks = [max(1, int(math.ceil(uniq_counts[:, b].max() / 128.0)))
                for b in range(nblk)]
    tile_off = np.concatenate([[0], np.cumsum(t_blocks)]).astype(np.int64)
    tot_tiles = int(tile_off[-1])
    tot_slots = tot_tiles * 128

    # per-call structure (shared across cores)
    calls = []                        # (block, tile_lo, n_tiles)
    for b in range(nblk):
        done = 0
        while done < t_blocks[b]:
            ct = min(CALL_CAP_TILES, t_blocks[b] - done)
            calls.append((b, done, ct))
            done += ct
    n_calls = len(calls)

    idxw_list, ind_list, cnt_list, dloc_list = [], [], [], []
    s_arange = np.arange(tot_slots)
    for c in range(n_cores):
        cc_, vc, gid, gsrc, gblk, cntb = percore[c]
        bstart = np.concatenate([[0], np.cumsum(cntb)])
        grank = np.arange(len(gsrc)) - bstart[gblk]
        gslot = tile_off[gblk] * 128 + grank
        eslot = gslot[gid]

        idx_slots = np.full(tot_slots, -1, dtype=np.int16)
        idx_slots[gslot] = gsrc.astype(np.int16)
        ind = np.zeros((128, tot_slots), dtype=np.float32)
        np.add.at(ind, (eslot % 128, (eslot // 128) * 128 + cc_), vc)

        cnts = []
        for b, tlo, ct in calls:
            lo = tlo * 128
            v = min(max(int(cntb[b]) - lo, 0), ct * 128)
            cnts.append(v)
        cnt_list.append(np.array(cnts, dtype=np.uint32)[None, :])

        idxw = np.zeros((128, tot_slots // 16), dtype=np.int16)
        idxw[s_arange % 16, s_arange // 16] = idx_slots
        for r in range(1, 8):
            idxw[16 * r: 16 * (r + 1)] = idxw[:16]
        idxw_list.append(idxw)
        ind_list.append(np.ascontiguousarray(ind.astype(ind_np_dtype)))

        # dense local chunks [dst block, src block]: self-loop diagonal +
        # all intra-core edges
        dloc = np.zeros((128, nblk * nblk * 128), dtype=np.float32)
        rows = np.arange(npc)
        rb = rows // 128
        dloc[rows % 128, (rb * nblk + rb) * 128 + rows % 128] = \
            1.0 / deg[c * npc + rows]
        ml = (core == c) & is_local
        np.add.at(
            dloc,
            (sl[ml] % 128, (blk[ml] * nblk + sl[ml] // 128) * 128 + col[ml]),
            norm[ml],
        )
        dloc_list.append(np.ascontiguousarray(dloc.astype(ind_np_dtype)))

    # ---- layer-2 dense structure in the permuted AllGather row space ----
    ns_tiles = (n_nodes + 127) // 128
    split_blocks = split_blocks or [nblk]
    if sum(split_blocks) != nblk:
        split_blocks = [nblk]
    grp_rows = []
    acc = 0
    for gnb in split_blocks:
        lo = acc * 128
        hi = min((acc + gnb) * 128, npc)
        grp_rows.append(hi - lo)
        acc += gnb
    grp_pos0 = np.concatenate([[0], np.cumsum([r * n_cores for r in grp_rows])])
    assert int(grp_pos0[-1]) == n_nodes

    # self-loops folded in as ordinary entries for the dense layer-2 matrix
    iota = np.arange(n_nodes, dtype=np.int64)
    s_all = np.concatenate([src, iota])
    d_all = np.concatenate([dst, iota])
    v_all = np.concatenate([norm, (1.0 / deg).astype(np.float32)])
    core_all = d_all // npc
    dl_all = d_all % npc
    blk_all = dl_all // 128
    col_all = dl_all % 128

    def pos_of(n):
        c = n // npc
        l = n % npc
        g = np.zeros_like(n)
        loc = l.copy()
        acc2 = 0
        for gi, gnb in enumerate(split_blocks):
            lo, hi = acc2 * 128, min((acc2 + gnb) * 128, npc)
            mask = (l >= lo) & (l < hi)
            g[mask] = gi
            loc[mask] = l[mask] - lo
            acc2 += gnb
        base = grp_pos0[g]
        rows = np.array(grp_rows)[g]
        return base + c * rows + loc

    ind2_list = []
    for c in range(n_cores):
        m = core_all == c
        sc, bc, cc2, vc = s_all[m], blk_all[m], col_all[m], v_all[m]
        pp = pos_of(sc)
        ind2 = np.zeros((128, nblk * ns_tiles * 128), dtype=np.float32)
        np.add.at(ind2, (pp % 128, (bc * ns_tiles + pp // 128) * 128 + cc2), vc)
        ind2_list.append(np.ascontiguousarray(ind2.astype(ind_np_dtype)))

    return dict(
        cnts=cnt_list,
        calls=calls,
        n_calls=n_calls,
        ns_tiles=ns_tiles,
        split_blocks=split_blocks,
        grp_rows=grp_rows,
        grp_pos0=[int(v) for v in grp_pos0],
        ind2=ind2_list,
        dloc=dloc_list,
        npc=npc,
        nblk=nblk,
        t_blocks=t_blocks,
        tile_off=tile_off,
        tot_tiles=tot_tiles,
        idxw=idxw_list,
        ind=ind_list,
    )


# --------------------------------------------------------------------------
# device kernel
# --------------------------------------------------------------------------
def _build_nc(n_nodes, f1, f2, f3, npc, nblk, t_blocks, tile_off, n_cores,
              compute_dtype="bf16", split_blocks=None, grp_rows=None,
              grp_pos0=None, calls=None, n_calls=0, ns_tiles=0):
    import concourse.mybir as mybir
    import concourse.tile as tile
    from concourse import bacc
    from concourse.masks import make_identity

    f32 = mybir.dt.float32
    i16 = mybir.dt.int16
    cdt = mybir.dt.bfloat16 if compute_dtype == "bf16" else mybir.dt.float32
    tot_tiles = int(tile_off[-1])
    tot_slots = tot_tiles * 128
    kf1, kf2 = f1 // 128, f2 // 128
    ngrp = len(split_blocks)

    nc = bacc.Bacc(num_devices=n_cores)
    x_ext = nc.declare_dram_parameter("x", [n_nodes, f1], cdt, isOutput=False)
    xloc_ext = nc.declare_dram_parameter("xloc", [nblk * 128, f1], cdt, isOutput=False)
    w1_ext = nc.declare_dram_parameter("w1", [f1, f2], cdt, isOutput=False)
    w2_ext = nc.declare_dram_parameter("w2", [f2, f3], cdt, isOutput=False)
    ind_ext = nc.declare_dram_parameter("ind", [128, tot_slots], cdt, isOutput=False)
    idx_ext = nc.declare_dram_parameter("idxw", [128, tot_slots // 16], i16, isOutput=False)
    dloc_ext = nc.declare_dram_parameter("dloc", [128, nblk * nblk * 128], cdt,
                                         isOutput=False)
    cnt_ext = nc.declare_dram_parameter("cnts", [1, n_calls], mybir.dt.uint32,
                                        isOutput=False)
    ind2_ext = nc.declare_dram_parameter(
        "ind2", [128, nblk * ns_tiles * 128], cdt, isOutput=False
    )
    out_ext = nc.declare_dram_parameter("out", [npc, f3], f32, isOutput=True)

    # group bookkeeping
    grp_end = []
    acc = 0
    for gnb in split_blocks:
        grp_end.append(acc + gnb - 1)
        acc += gnb
    grp_of_block = []
    acc = 0
    for g, gnb in enumerate(split_blocks):
        grp_of_block += [g] * gnb
        acc += gnb
    grp_b0 = [0]
    for gnb in split_blocks[:-1]:
        grp_b0.append(grp_b0[-1] + gnb)
    grp_tile0 = [p0 // 128 for p0 in grp_pos0]

    with tile.TileContext(nc) as tc:
        with tc.tile_pool(name="dram", bufs=1, space="DRAM") as dpool, \
             tc.tile_pool(name="const", bufs=1) as cpool, \
             tc.tile_pool(name="gbp", bufs=4) as gpool, \
             tc.tile_pool(name="i2p", bufs=3) as i2pool, \
             tc.tile_pool(name="dlp", bufs=2) as dlpool, \
             tc.tile_pool(name="work", bufs=2) as wpool, \
             tc.tile_pool(name="psagg", bufs=2, space="PSUM") as ps_agg_p, \
             tc.tile_pool(name="pstr", bufs=1, space="PSUM") as ps_tr_p, \
             tc.tile_pool(name="psc1", bufs=1, space="PSUM") as ps_c1_p, \
             tc.tile_pool(name="psh2", bufs=1, space="PSUM") as ps_h2_p, \
             tc.tile_pool(name="pso", bufs=3, space="PSUM") as ps_o_p:

            cc_in_g = [
                dpool.tile([grp_rows[g], f3], cdt, name=f"ccin{g}")
                for g in range(ngrp)
            ]
            h2p_g = [
                dpool.tile([grp_rows[g] * n_cores, f3], cdt,
                           addr_space="Shared", name=f"h2p{g}")
                for g in range(ngrp)
            ]

            # warm the CC stream so the first real AllGather has no pickup
            # latency (first collective pays ~11us otherwise)
            warm_in = dpool.tile([128, 16], cdt, name="warm_in")
            warm_out = dpool.tile([128 * n_cores, 16], cdt, name="warm_out")
            nc.gpsimd.collective_compute(
                "AllGather",
                mybir.AluOpType.bypass,
                replica_groups=[list(range(n_cores))],
                ins=[warm_in[:, :].opt()],
                outs=[warm_out[:, :].opt()],
            )

            # ---- startup loads (ordered so block-0 work starts ASAP) ----
            cnt_sb = cpool.tile([1, n_calls], mybir.dt.uint32)
            nc.sync.dma_start(out=cnt_sb[:, :], in_=cnt_ext[:, :])
            idx_sb = []
            ind_sb = []
            for b in range(nblk):
                tb, t0 = t_blocks[b], int(tile_off[b])
                ix = cpool.tile([128, tb * 8], i16, name=f"idx{b}")
                nc.sync.dma_start(out=ix[:, :], in_=idx_ext[:, t0 * 8: (t0 + tb) * 8])
                idx_sb.append(ix)
            xloc_sb = cpool.tile([128, nblk, f1], cdt)
            nc.scalar.dma_start(
                out=xloc_sb[:, :, :],
                in_=xloc_ext[:, :].rearrange("(b p) f -> p b f", p=128),
            )
            for b in range(nblk):
                tb, t0 = t_blocks[b], int(tile_off[b])
                iv = cpool.tile([128, tb * 128], cdt, name=f"ind{b}")
                nc.sync.dma_start(out=iv[:, :],
                                  in_=ind_ext[:, t0 * 128: (t0 + tb) * 128])
                ind_sb.append(iv)

            w1_sb = cpool.tile([128, kf1 * f2], cdt)  # chunk (k,m) at (k*kf2+m)*128
            for k in range(kf1):
                for m_ in range(kf2):
                    nc.scalar.dma_start(
                        out=w1_sb[:, (k * kf2 + m_) * 128: (k * kf2 + m_ + 1) * 128],
                        in_=w1_ext[k * 128: (k + 1) * 128, m_ * 128: (m_ + 1) * 128],
                    )
            w2_sb = cpool.tile([128, kf2 * f3], cdt)
            for k in range(kf2):
                nc.scalar.dma_start(
                    out=w2_sb[:, k * f3: (k + 1) * f3],
                    in_=w2_ext[k * 128: (k + 1) * 128, :],
                )
            ident = cpool.tile([128, 128], cdt)
            make_identity(nc, ident)

            # rotating count registers to avoid a register hazard between
            # consecutive gather calls
            cnt_regs = [nc.gpsimd.to_reg(0) for _ in range(4)]
            call_i = [0]

            # gather buffers are zeroed lazily at first use (pad slots must
            # read as finite values): call k waits only its own buffer's
            # memset, which Vector completes during earlier calls, instead of
            # all four memsets serializing before the first gather
            gbuf_zeroed = [0]

            # L2 state
            h2f_g = [None] * ngrp
            part_sb = {}

            def emit_cc(g):
                nc.gpsimd.collective_compute(
                    "AllGather",
                    mybir.AluOpType.bypass,
                    replica_groups=[list(range(n_cores))],
                    ins=[cc_in_g[g][:, :].opt()],
                    outs=[h2p_g[g][:, :].opt()],
                )

            def emit_h2f_load(g):
                gpos = grp_rows[g] * n_cores
                gt = (gpos + 127) // 128
                h2f = cpool.tile([128, gt, f3], cdt, name=f"h2f{g}")
                fullt = gpos // 128
                rem = gpos - fullt * 128
                h2v = h2p_g[g][: fullt * 128, :].rearrange("(s p) f -> p s f", p=128)
                for s0_ in range(0, fullt, 6):
                    s1_ = min(s0_ + 6, fullt)
                    nc.sync.dma_start(out=h2f[:, s0_:s1_, :], in_=h2v[:, s0_:s1_, :])
                if rem:
                    nc.vector.memset(h2f[:, fullt, :], 0.0)
                    nc.sync.dma_start(
                        out=h2f[:rem, fullt, :], in_=h2p_g[g][fullt * 128:, :]
                    )
                h2f_g[g] = h2f

            def emit_l2(g, b_range):
                gpos = grp_rows[g] * n_cores
                gt = (gpos + 127) // 128
                s0 = grp_tile0[g]
                for b in b_range:
                    nb = min(128, npc - b * 128)
                    i2 = i2pool.tile([128, gt * 128], cdt, tag="i2")
                    nc.scalar.dma_start(
                        out=i2[:, :],
                        in_=ind2_ext[
                            :, (b * ns_tiles + s0) * 128: (b * ns_tiles + s0 + gt) * 128
                        ],
                    )
                    ps_o = ps_o_p.tile([128, f3], f32, tag="o")
                    for s in range(gt):
                        nc.tensor.matmul(
                            ps_o[:, :],
                            lhsT=i2[:, s * 128: (s + 1) * 128],
                            rhs=h2f_g[g][:, s, :],
                            start=(s == 0),
                            stop=(s == gt - 1),
                        )
                    if g == 0:
                        pt = wpool.tile([128, f3], f32, tag=f"part{b}", bufs=1,
                                        name=f"part{b}")
                        nc.scalar.copy(pt[:, :], ps_o[:, :])
                        part_sb[b] = pt
                    elif g < ngrp - 1:
                        nc.vector.tensor_tensor(
                            out=part_sb[b][:, :],
                            in0=part_sb[b][:, :],
                            in1=ps_o[:, :],
                            op=mybir.AluOpType.add,
                        )
                    else:
                        o_sb = wpool.tile([128, f3], f32, tag="osb")
                        nc.vector.tensor_tensor(
                            out=o_sb[:, :],
                            in0=part_sb[b][:, :],
                            in1=ps_o[:, :],
                            op=mybir.AluOpType.add,
                        )
                        nc.sync.dma_start(
                            out=out_ext[b * 128: b * 128 + nb, :], in_=o_sb[:nb, :]
                        )

            # ---------------- layer 1 (with interleaved layer-2 work) -------
            for b in range(nblk):
                nb = min(128, npc - b * 128)
                tb = t_blocks[b]

                ps_agg = ps_agg_p.tile([128, f1], f32, tag="agg")
                # dense local chunks: self loops + all intra-core edges, using
                # the SBUF-resident local x rows (no gather descriptors)
                dl2 = dlpool.tile([128, nblk * 128], cdt, tag="dl")
                nc.scalar.dma_start(
                    out=dl2[:, :],
                    in_=dloc_ext[:, b * nblk * 128: (b + 1) * nblk * 128],
                )
                for sb in range(nblk):
                    nc.tensor.matmul(
                        ps_agg[:, :],
                        lhsT=dl2[:, sb * 128: (sb + 1) * 128],
                        rhs=xloc_sb[:, sb, :],
                        start=(sb == 0),
                        stop=False,
                    )
                done = 0
                while done < tb:
                    ct = min(CALL_CAP_TILES, tb - done)
                    gb = gpool.tile([128, CALL_CAP_TILES, f1], cdt, tag="gbuf")
                    if gbuf_zeroed[0] < 4:
                        nc.vector.memset(gb[:, :, :], 0.0)
                        gbuf_zeroed[0] += 1
                    reg = cnt_regs[call_i[0] % 4]
                    nc.gpsimd.reg_load(reg, cnt_sb[0:1, call_i[0]: call_i[0] + 1])
                    call_i[0] += 1
                    nc.gpsimd.dma_gather(
                        out_ap=gb[:, :ct, :],
                        in_ap=x_ext[:, :],
                        idxs_ap=idx_sb[b][:, done * 8: (done + ct) * 8],
                        num_idxs=ct * 128,
                        num_idxs_reg=reg,
                        elem_size=f1,
                    )
                    for t in range(ct):
                        tt = done + t
                        nc.tensor.matmul(
                            ps_agg[:, :],
                            lhsT=ind_sb[b][:, tt * 128: (tt + 1) * 128],
                            rhs=gb[:, t, :],
                            start=False,
                            stop=(tt == tb - 1),
                        )
                    done += ct

                # dispatch the previous group's AllGather here, after this
                # block's gathers: by the time GpSimd reaches the trigger the
                # group's last cc_in write has completed, so the trigger does
                # not stall GpSimd (head-of-line) and the gathers stay packed
                for g in range(ngrp):
                    if grp_end[g] == b - 1:
                        emit_cc(g)

                agg_sb = wpool.tile([128, f1], cdt, tag="agg_sb")
                nc.vector.tensor_copy(agg_sb[:, :], ps_agg[:, :])
                ps_tr = ps_tr_p.tile([128, f1], cdt, tag="tr")
                for k in range(kf1):
                    nc.tensor.transpose(
                        ps_tr[:, k * 128: (k + 1) * 128],
                        agg_sb[:, k * 128: (k + 1) * 128],
                        ident,
                    )
                aggT_sb = wpool.tile([128, f1], cdt, tag="aggT")
                nc.vector.tensor_copy(aggT_sb[:, :], ps_tr[:, :])

                ps_c1 = ps_c1_p.tile([128, f2], f32, tag="c1")
                firstmm = True
                for m_ in range(kf2):
                    for k in range(kf1):
                        nc.tensor.matmul(
                            ps_c1[:, m_ * 128: (m_ + 1) * 128],
                            lhsT=w1_sb[:, (k * kf2 + m_) * 128: (k * kf2 + m_ + 1) * 128],
                            rhs=aggT_sb[:, k * 128: (k + 1) * 128],
                            start=firstmm,
                            stop=(m_ == kf2 - 1 and k == kf1 - 1),
                        )
                        firstmm = False
                h1T_sb = wpool.tile([128, f2], cdt, tag="h1T")
                nc.scalar.activation(
                    h1T_sb[:, :], ps_c1[:, :], mybir.ActivationFunctionType.Relu
                )
                ps_h2 = ps_h2_p.tile([128, f3], f32, tag="h2")
                for k in range(kf2):
                    nc.tensor.matmul(
                        ps_h2[:, :],
                        lhsT=h1T_sb[:, k * 128: (k + 1) * 128],
                        rhs=w2_sb[:, k * f3: (k + 1) * f3],
                        start=(k == 0),
                        stop=(k == kf2 - 1),
                    )
                h2_sb = wpool.tile([128, f3], cdt, tag="h2sb")
                nc.scalar.copy(h2_sb[:, :], ps_h2[:, :])
                g = grp_of_block[b]
                off = (b - grp_b0[g]) * 128
                nc.sync.dma_start(
                    out=cc_in_g[g][off: off + nb, :], in_=h2_sb[:nb, :]
                )

                # interleave layer-2 group 0 in small chunks once its
                # AllGather has certainly completed, so TensorE fills its
                # layer-1 idle time without delaying the later blocks'
                # transforms (which gate the remaining AllGathers)
                if b == nblk - 4:
                    emit_h2f_load(0)
                    emit_l2(0, range(0, 3))
                elif b == nblk - 3:
                    emit_l2(0, range(3, 6))
                elif b == nblk - 2:
                    emit_l2(0, range(6, 8))
                elif b == nblk - 1:
                    emit_h2f_load(1)
                    emit_l2(0, range(8, nblk))

            # ---------------- remaining collectives + layer 2 ---------------
            for g in range(ngrp):
                if grp_end[g] == nblk - 1:
                    emit_cc(g)
            for g in range(1, ngrp):
                if h2f_g[g] is None:
                    emit_h2f_load(g)
                emit_l2(g, range(nblk))

    nc.finalize()
    return nc


def _make_in_maps(x, W1, W2, g, n_cores, npc, nblk):
    maps = []
    for c in range(n_cores):
        xloc = np.zeros((nblk * 128, x.shape[1]), dtype=x.dtype)
        xloc[:npc] = x[c * npc: (c + 1) * npc]
        maps.append({
            "x": x,
            "xloc": np.ascontiguousarray(xloc),
            "w1": W1,
            "w2": W2,
            "ind": g["ind"][c],
            "idxw": g["idxw"][c],
            "dloc": g["dloc"][c],
            "ind2": g["ind2"][c],
            "cnts": g["cnts"][c],
        })
    return maps


def build_all(x, edge_index, edge_weight, W1, W2, n_cores=N_CORES,
              compute_dtype=COMPUTE_DTYPE):
    """Host packing + Bass graph for the given full inputs."""
    if compute_dtype == "bf16":
        import ml_dtypes

        np_cdt = ml_dtypes.bfloat16
    else:
        np_cdt = np.float32
    x = np.ascontiguousarray(np.asarray(x, dtype=np.float32).astype(np_cdt))
    W1 = np.ascontiguousarray(np.asarray(W1, dtype=np.float32).astype(np_cdt))
    W2 = np.ascontiguousarray(np.asarray(W2, dtype=np.float32).astype(np_cdt))
    n_nodes, f1 = x.shape
    f2, f3 = W1.shape[1], W2.shape[1]
    g = _pack_graph(edge_index, edge_weight, n_nodes, n_cores,
                    ind_np_dtype=np_cdt, split_blocks=SPLIT_BLOCKS)
    nc = _build_nc(
        n_nodes, f1, f2, f3, g["npc"], g["nblk"], g["t_blocks"], g["tile_off"],
        n_cores, compute_dtype=compute_dtype, split_blocks=g["split_blocks"],
        grp_rows=g["grp_rows"], grp_pos0=g["grp_pos0"], calls=g["calls"],
        n_calls=g["n_calls"], ns_tiles=g["ns_tiles"],
    )
    return nc, _make_in_maps(x, W1, W2, g, n_cores, g["npc"], g["nblk"]), g


def kernel(x, edge_index, edge_weight, W1, W2):
    from concourse.bass_utils import run_bass_kernel_spmd

    nc, in_maps, _ = build_all(x, edge_index, edge_weight, W1, W2)
    res = run_bass_kernel_spmd(nc, in_maps, list(range(N_CORES)))
    out = np.concatenate(
        [np.asarray(res.results[c]["out"]) for c in range(N_CORES)], axis=0
    )
    return out.astype(np.float32)



# revision 13
# speedup vs baseline: 1.0054x; 1.0054x over previous
"""2-layer GCN (PyG GCNConv, bias=False, normalize=True) on 8 TRN2 NeuronCores.

Math: out = A @ relu(A @ X @ W1) @ W2 with A = D^{-1/2} (A_w + I) D^{-1/2}.
Layer 1 is (A@X)@W1 against replicated X (zero communication); layer 2 is
A@(h1@W2) with the small per-core H2 = h1@W2 shard exchanged via 4 grouped
AllGathers that overlap layer-1 compute.

Sharding: destination nodes are block-partitioned across the 8 cores.  The
SWDGE gather ucode on GpSimd (~7ns per gathered row, serial) is the pacing
engine for layer 1, so the packing minimizes gather descriptors: edges into
each 128-row dst block are DEDUPLICATED by source node (a gathered source
row feeds all its dst columns through one multi-hot indicator column), and
self-loops plus ALL intra-core edges are folded into dense per-(dst block,
src block) [128 x 128] chunks multiplied against the core's SBUF-resident
local x rows (no descriptors at all).  Layer 2 is fully dense per
(dst block, src tile) in the permuted AllGather row space, and its TensorE
work is interleaved into the layer-1 emission (group 0 under layer 1 once
its AllGather completes; later groups immediately after) so the PE array
stays busy while GpSimd paces layer 1.
"""

import math

import numpy as np

N_CORES = 8
COMPUTE_DTYPE = "bf16"  # "f32" or "bf16"
SPLIT_BLOCKS = [3, 3, 3, 1]  # dst-block groups per collective split
CALL_CAP_TILES = 8  # idxs per call capped by the SWDGE descriptor ring


# --------------------------------------------------------------------------
# host-side graph packing
# --------------------------------------------------------------------------
def _pack_graph(edge_index, edge_weight, n_nodes, n_cores, ind_np_dtype=np.float32,
                split_blocks=None):
    src = np.asarray(edge_index[0], dtype=np.int64)
    dst = np.asarray(edge_index[1], dtype=np.int64)
    w = np.asarray(edge_weight, dtype=np.float32)

    deg = np.zeros(n_nodes, dtype=np.float32)
    np.add.at(deg, dst, w)
    deg += np.float32(1.0)
    dinv = (1.0 / np.sqrt(deg)).astype(np.float32)
    norm = (dinv[src] * w * dinv[dst]).astype(np.float32)

    npc = n_nodes // n_cores          # nodes per core
    nblk = (npc + 127) // 128         # dst blocks per core

    core = dst // npc
    dl = dst % npc
    blk = dl // 128
    col = dl % 128                    # indicator column within block
    score = src // npc
    sl = src % npc
    # edges whose src lies on the same core as their dst are handled by dense
    # per-(dst block, src block) chunks against the SBUF-resident local x rows
    # (together with the self loops), skipping the gather for them entirely
    is_local = score == core

    # ---- pass 1: per-core per-block unique-source counts (tile structure
    # must be SPMD-uniform across cores) ----
    percore = []
    uniq_counts = np.zeros((n_cores, nblk), dtype=np.int64)
    for c in range(n_cores):
        m = (core == c) & ~is_local
        sc, bc, cc_, vc = src[m], blk[m], col[m], norm[m]
        order = np.lexsort((sc, bc))
        sc, bc, cc_, vc = sc[order], bc[order], cc_[order], vc[order]
        new = np.ones(len(sc), dtype=bool)
        if len(sc) > 1:
            new[1:] = (sc[1:] != sc[:-1]) | (bc[1:] != bc[:-1])
        gid = np.cumsum(new) - 1          # unique (blk, src) group per edge
        gsrc, gblk = sc[new], bc[new]
        cntb = np.bincount(gblk, minlength=nblk)
        uniq_counts[c] = cntb
        percore.append((cc_, vc, gid, gsrc, gblk, cntb))

    t_blocks = [max(1, int(math.ceil(uniq_counts[:, b].max() / 128.0)))
                for b in range(nblk)]
    tile_off = np.concatenate([[0], np.cumsum(t_blocks)]).astype(np.int64)
    tot_tiles = int(tile_off[-1])
    tot_slots = tot_tiles * 128

    # per-call structure (shared across cores)
    calls = []                        # (block, tile_lo, n_tiles)
    for b in range(nblk):
        done = 0
        while done < t_blocks[b]:
            ct = min(CALL_CAP_TILES, t_blocks[b] - done)
            calls.append((b, done, ct))
            done += ct
    n_calls = len(calls)

    idxw_list, ind_list, cnt_list, dloc_list = [], [], [], []
    s_arange = np.arange(tot_slots)
    for c in range(n_cores):
        cc_, vc, gid, gsrc, gblk, cntb = percore[c]
        bstart = np.concatenate([[0], np.cumsum(cntb)])
        grank = np.arange(len(gsrc)) - bstart[gblk]
        gslot = tile_off[gblk] * 128 + grank
        eslot = gslot[gid]

        idx_slots = np.full(tot_slots, -1, dtype=np.int16)
        idx_slots[gslot] = gsrc.astype(np.int16)
        ind = np.zeros((128, tot_slots), dtype=np.float32)
        np.add.at(ind, (eslot % 128, (eslot // 128) * 128 + cc_), vc)

        cnts = []
        for b, tlo, ct in calls:
            lo = tlo * 128
            v = min(max(int(cntb[b]) - lo, 0), ct * 128)
            cnts.append(v)
        cnt_list.append(np.array(cnts, dtype=np.uint32)[None, :])

        idxw = np.zeros((128, tot_slots // 16), dtype=np.int16)
        idxw[s_arange % 16, s_arange // 16] = idx_slots
        for r in range(1, 8):
            idxw[16 * r: 16 * (r + 1)] = idxw[:16]
        idxw_list.append(idxw)
        ind_list.append(np.ascontiguousarray(ind.astype(ind_np_dtype)))

        # dense local chunks [dst block, src block]: self-loop diagonal +
        # all intra-core edges
        dloc = np.zeros((128, nblk * nblk * 128), dtype=np.float32)
        rows = np.arange(npc)
        rb = rows // 128
        dloc[rows % 128, (rb * nblk + rb) * 128 + rows % 128] = \
            1.0 / deg[c * npc + rows]
        ml = (core == c) & is_local
        np.add.at(
            dloc,
            (sl[ml] % 128, (blk[ml] * nblk + sl[ml] // 128) * 128 + col[ml]),
            norm[ml],
        )
        dloc_list.append(np.ascontiguousarray(dloc.astype(ind_np_dtype)))

    # ---- layer-2 dense structure in the permuted AllGather row space ----
    ns_tiles = (n_nodes + 127) // 128
    split_blocks = split_blocks or [nblk]
    if sum(split_blocks) != nblk:
        split_blocks = [nblk]
    grp_rows = []
    acc = 0
    for gnb in split_blocks:
        lo = acc * 128
        hi = min((acc + gnb) * 128, npc)
        grp_rows.append(hi - lo)
        acc += gnb
    grp_pos0 = np.concatenate([[0], np.cumsum([r * n_cores for r in grp_rows])])
    assert int(grp_pos0[-1]) == n_nodes

    # self-loops folded in as ordinary entries for the dense layer-2 matrix
    iota = np.arange(n_nodes, dtype=np.int64)
    s_all = np.concatenate([src, iota])
    d_all = np.concatenate([dst, iota])
    v_all = np.concatenate([norm, (1.0 / deg).astype(np.float32)])
    core_all = d_all // npc
    dl_all = d_all % npc
    blk_all = dl_all // 128
    col_all = dl_all % 128

    def pos_of(n):
        c = n // npc
        l = n % npc
        g = np.zeros_like(n)
        loc = l.copy()
        acc2 = 0
        for gi, gnb in enumerate(split_blocks):
            lo, hi = acc2 * 128, min((acc2 + gnb) * 128, npc)
            mask = (l >= lo) & (l < hi)
            g[mask] = gi
            loc[mask] = l[mask] - lo
            acc2 += gnb
        base = grp_pos0[g]
        rows = np.array(grp_rows)[g]
        return base + c * rows + loc

    ind2_list = []
    for c in range(n_cores):
        m = core_all == c
        sc, bc, cc2, vc = s_all[m], blk_all[m], col_all[m], v_all[m]
        pp = pos_of(sc)
        ind2 = np.zeros((128, nblk * ns_tiles * 128), dtype=np.float32)
        np.add.at(ind2, (pp % 128, (bc * ns_tiles + pp // 128) * 128 + cc2), vc)
        ind2_list.append(np.ascontiguousarray(ind2.astype(ind_np_dtype)))

    return dict(
        cnts=cnt_list,
        calls=calls,
        n_calls=n_calls,
        ns_tiles=ns_tiles,
        split_blocks=split_blocks,
        grp_rows=grp_rows,
        grp_pos0=[int(v) for v in grp_pos0],
        ind2=ind2_list,
        dloc=dloc_list,
        npc=npc,
        nblk=nblk,
        t_blocks=t_blocks,
        tile_off=tile_off,
        tot_tiles=tot_tiles,
        idxw=idxw_list,
        ind=ind_list,
    )


# --------------------------------------------------------------------------
# device kernel
# --------------------------------------------------------------------------
def _build_nc(n_nodes, f1, f2, f3, npc, nblk, t_blocks, tile_off, n_cores,
              compute_dtype="bf16", split_blocks=None, grp_rows=None,
              grp_pos0=None, calls=None, n_calls=0, ns_tiles=0):
    import concourse.mybir as mybir
    import concourse.tile as tile
    from concourse import bacc
    from concourse.masks import make_identity

    f32 = mybir.dt.float32
    i16 = mybir.dt.int16
    cdt = mybir.dt.bfloat16 if compute_dtype == "bf16" else mybir.dt.float32
    tot_tiles = int(tile_off[-1])
    tot_slots = tot_tiles * 128
    kf1, kf2 = f1 // 128, f2 // 128
    ngrp = len(split_blocks)

    nc = bacc.Bacc(num_devices=n_cores)
    x_ext = nc.declare_dram_parameter("x", [n_nodes, f1], cdt, isOutput=False)
    xloc_ext = nc.declare_dram_parameter("xloc", [nblk * 128, f1], cdt, isOutput=False)
    w1_ext = nc.declare_dram_parameter("w1", [f1, f2], cdt, isOutput=False)
    w2_ext = nc.declare_dram_parameter("w2", [f2, f3], cdt, isOutput=False)
    ind_ext = nc.declare_dram_parameter("ind", [128, tot_slots], cdt, isOutput=False)
    idx_ext = nc.declare_dram_parameter("idxw", [128, tot_slots // 16], i16, isOutput=False)
    dloc_ext = nc.declare_dram_parameter("dloc", [128, nblk * nblk * 128], cdt,
                                         isOutput=False)
    cnt_ext = nc.declare_dram_parameter("cnts", [1, n_calls], mybir.dt.uint32,
                                        isOutput=False)
    ind2_ext = nc.declare_dram_parameter(
        "ind2", [128, nblk * ns_tiles * 128], cdt, isOutput=False
    )
    out_ext = nc.declare_dram_parameter("out", [npc, f3], f32, isOutput=True)

    # group bookkeeping
    grp_end = []
    acc = 0
    for gnb in split_blocks:
        grp_end.append(acc + gnb - 1)
        acc += gnb
    grp_of_block = []
    acc = 0
    for g, gnb in enumerate(split_blocks):
        grp_of_block += [g] * gnb
        acc += gnb
    grp_b0 = [0]
    for gnb in split_blocks[:-1]:
        grp_b0.append(grp_b0[-1] + gnb)
    grp_tile0 = [p0 // 128 for p0 in grp_pos0]

    with tile.TileContext(nc) as tc:
        with tc.tile_pool(name="dram", bufs=1, space="DRAM") as dpool, \
             tc.tile_pool(name="const", bufs=1) as cpool, \
             tc.tile_pool(name="gbp", bufs=4) as gpool, \
             tc.tile_pool(name="i2p", bufs=3) as i2pool, \
             tc.tile_pool(name="dlp", bufs=2) as dlpool, \
             tc.tile_pool(name="work", bufs=2) as wpool, \
             tc.tile_pool(name="psagg", bufs=2, space="PSUM") as ps_agg_p, \
             tc.tile_pool(name="pstr", bufs=1, space="PSUM") as ps_tr_p, \
             tc.tile_pool(name="psc1", bufs=1, space="PSUM") as ps_c1_p, \
             tc.tile_pool(name="psh2", bufs=1, space="PSUM") as ps_h2_p, \
             tc.tile_pool(name="pso", bufs=3, space="PSUM") as ps_o_p:

            cc_in_g = [
                dpool.tile([grp_rows[g], f3], cdt, name=f"ccin{g}")
                for g in range(ngrp)
            ]
            h2p_g = [
                dpool.tile([grp_rows[g] * n_cores, f3], cdt,
                           addr_space="Shared", name=f"h2p{g}")
                for g in range(ngrp)
            ]

            # warm the CC stream so the first real AllGather has no pickup
            # latency (first collective pays ~11us otherwise)
            warm_in = dpool.tile([128, 16], cdt, name="warm_in")
            warm_out = dpool.tile([128 * n_cores, 16], cdt, name="warm_out")
            nc.gpsimd.collective_compute(
                "AllGather",
                mybir.AluOpType.bypass,
                replica_groups=[list(range(n_cores))],
                ins=[warm_in[:, :].opt()],
                outs=[warm_out[:, :].opt()],
            )

            # ---- startup loads (ordered so block-0 work starts ASAP) ----
            cnt_sb = cpool.tile([1, n_calls], mybir.dt.uint32)
            nc.sync.dma_start(out=cnt_sb[:, :], in_=cnt_ext[:, :])
            idx_sb = []
            ind_sb = []
            for b in range(nblk):
                tb, t0 = t_blocks[b], int(tile_off[b])
                ix = cpool.tile([128, tb * 8], i16, name=f"idx{b}")
                nc.sync.dma_start(out=ix[:, :], in_=idx_ext[:, t0 * 8: (t0 + tb) * 8])
                idx_sb.append(ix)
            xloc_sb = cpool.tile([128, nblk, f1], cdt)
            nc.scalar.dma_start(
                out=xloc_sb[:, :, :],
                in_=xloc_ext[:, :].rearrange("(b p) f -> p b f", p=128),
            )
            for b in range(nblk):
                tb, t0 = t_blocks[b], int(tile_off[b])
                iv = cpool.tile([128, tb * 128], cdt, name=f"ind{b}")
                nc.sync.dma_start(out=iv[:, :],
                                  in_=ind_ext[:, t0 * 128: (t0 + tb) * 128])
                ind_sb.append(iv)

            w1_sb = cpool.tile([128, kf1 * f2], cdt)  # chunk (k,m) at (k*kf2+m)*128
            for k in range(kf1):
                for m_ in range(kf2):
                    nc.scalar.dma_start(
                        out=w1_sb[:, (k * kf2 + m_) * 128: (k * kf2 + m_ + 1) * 128],
                        in_=w1_ext[k * 128: (k + 1) * 128, m_ * 128: (m_ + 1) * 128],
                    )
            w2_sb = cpool.tile([128, kf2 * f3], cdt)
            for k in range(kf2):
                nc.scalar.dma_start(
                    out=w2_sb[:, k * f3: (k + 1) * f3],
                    in_=w2_ext[k * 128: (k + 1) * 128, :],
                )
            ident = cpool.tile([128, 128], cdt)
            make_identity(nc, ident)

            # rotating count registers to avoid a register hazard between
            # consecutive gather calls
            cnt_regs = [nc.gpsimd.to_reg(0) for _ in range(4)]
            call_i = [0]

            # zero the gather buffers once, before the loop, so pad slots read
            # as finite values (0 * NaN = NaN) without chaining each block's
            # gather to the previous block's compute through an in-loop memset
            for _ in range(4):
                gb0 = gpool.tile([128, CALL_CAP_TILES, f1], cdt, tag="gbuf")
                nc.vector.memset(gb0[:, :, :], 0.0)

            # L2 state
            h2f_g = [None] * ngrp
            part_sb = {}

            def emit_cc(g):
                nc.gpsimd.collective_compute(
                    "AllGather",
                    mybir.AluOpType.bypass,
                    replica_groups=[list(range(n_cores))],
                    ins=[cc_in_g[g][:, :].opt()],
                    outs=[h2p_g[g][:, :].opt()],
                )

            def emit_h2f_load(g):
                gpos = grp_rows[g] * n_cores
                gt = (gpos + 127) // 128
                h2f = cpool.tile([128, gt, f3], cdt, name=f"h2f{g}")
                fullt = gpos // 128
                rem = gpos - fullt * 128
                h2v = h2p_g[g][: fullt * 128, :].rearrange("(s p) f -> p s f", p=128)
                for s0_ in range(0, fullt, 6):
                    s1_ = min(s0_ + 6, fullt)
                    nc.sync.dma_start(out=h2f[:, s0_:s1_, :], in_=h2v[:, s0_:s1_, :])
                if rem:
                    nc.vector.memset(h2f[:, fullt, :], 0.0)
                    nc.sync.dma_start(
                        out=h2f[:rem, fullt, :], in_=h2p_g[g][fullt * 128:, :]
                    )
                h2f_g[g] = h2f

            def emit_l2(g, b_range):
                gpos = grp_rows[g] * n_cores
                gt = (gpos + 127) // 128
                s0 = grp_tile0[g]
                for b in b_range:
                    nb = min(128, npc - b * 128)
                    i2 = i2pool.tile([128, gt * 128], cdt, tag="i2")
                    nc.scalar.dma_start(
                        out=i2[:, :],
                        in_=ind2_ext[
                            :, (b * ns_tiles + s0) * 128: (b * ns_tiles + s0 + gt) * 128
                        ],
                    )
                    ps_o = ps_o_p.tile([128, f3], f32, tag="o")
                    for s in range(gt):
                        nc.tensor.matmul(
                            ps_o[:, :],
                            lhsT=i2[:, s * 128: (s + 1) * 128],
                            rhs=h2f_g[g][:, s, :],
                            start=(s == 0),
                            stop=(s == gt - 1),
                        )
                    if g == 0:
                        pt = wpool.tile([128, f3], f32, tag=f"part{b}", bufs=1,
                                        name=f"part{b}")
                        nc.scalar.copy(pt[:, :], ps_o[:, :])
                        part_sb[b] = pt
                    elif g < ngrp - 1:
                        nc.vector.tensor_tensor(
                            out=part_sb[b][:, :],
                            in0=part_sb[b][:, :],
                            in1=ps_o[:, :],
                            op=mybir.AluOpType.add,
                        )
                    else:
                        o_sb = wpool.tile([128, f3], f32, tag="osb")
                        nc.vector.tensor_tensor(
                            out=o_sb[:, :],
                            in0=part_sb[b][:, :],
                            in1=ps_o[:, :],
                            op=mybir.AluOpType.add,
                        )
                        nc.sync.dma_start(
                            out=out_ext[b * 128: b * 128 + nb, :], in_=o_sb[:nb, :]
                        )

            # ---------------- layer 1 (with interleaved layer-2 work) -------
            for b in range(nblk):
                nb = min(128, npc - b * 128)
                tb = t_blocks[b]

                ps_agg = ps_agg_p.tile([128, f1], f32, tag="agg")
                # dense local chunks: self loops + all intra-core edges, using
                # the SBUF-resident local x rows (no gather descriptors)
                dl2 = dlpool.tile([128, nblk * 128], cdt, tag="dl")
                nc.scalar.dma_start(
                    out=dl2[:, :],
                    in_=dloc_ext[:, b * nblk * 128: (b + 1) * nblk * 128],
                )
                for sb in range(nblk):
                    nc.tensor.matmul(
                        ps_agg[:, :],
                        lhsT=dl2[:, sb * 128: (sb + 1) * 128],
                        rhs=xloc_sb[:, sb, :],
                        start=(sb == 0),
                        stop=False,
                    )
                done = 0
                while done < tb:
                    ct = min(CALL_CAP_TILES, tb - done)
                    gb = gpool.tile([128, CALL_CAP_TILES, f1], cdt, tag="gbuf")
                    reg = cnt_regs[call_i[0] % 4]
                    nc.gpsimd.reg_load(reg, cnt_sb[0:1, call_i[0]: call_i[0] + 1])
                    call_i[0] += 1
                    nc.gpsimd.dma_gather(
                        out_ap=gb[:, :ct, :],
                        in_ap=x_ext[:, :],
                        idxs_ap=idx_sb[b][:, done * 8: (done + ct) * 8],
                        num_idxs=ct * 128,
                        num_idxs_reg=reg,
                        elem_size=f1,
                    )
                    for t in range(ct):
                        tt = done + t
                        nc.tensor.matmul(
                            ps_agg[:, :],
                            lhsT=ind_sb[b][:, tt * 128: (tt + 1) * 128],
                            rhs=gb[:, t, :],
                            start=False,
                            stop=(tt == tb - 1),
                        )
                    done += ct

                # dispatch the previous group's AllGather here, after this
                # block's gathers: by the time GpSimd reaches the trigger the
                # group's last cc_in write has completed, so the trigger does
                # not stall GpSimd (head-of-line) and the gathers stay packed
                for g in range(ngrp):
                    if grp_end[g] == b - 1:
                        emit_cc(g)

                agg_sb = wpool.tile([128, f1], cdt, tag="agg_sb")
                nc.vector.tensor_copy(agg_sb[:, :], ps_agg[:, :])
                ps_tr = ps_tr_p.tile([128, f1], cdt, tag="tr")
                for k in range(kf1):
                    nc.tensor.transpose(
                        ps_tr[:, k * 128: (k + 1) * 128],
                        agg_sb[:, k * 128: (k + 1) * 128],
                        ident,
                    )
                aggT_sb = wpool.tile([128, f1], cdt, tag="aggT")
                nc.vector.tensor_copy(aggT_sb[:, :], ps_tr[:, :])

                ps_c1 = ps_c1_p.tile([128, f2], f32, tag="c1")
                firstmm = True
                for m_ in range(kf2):
                    for k in range(kf1):
                        nc.tensor.matmul(
                            ps_c1[:, m_ * 128: (m_ + 1) * 128],
                            lhsT=w1_sb[:, (k * kf2 + m_) * 128: (k * kf2 + m_ + 1) * 128],
                            rhs=aggT_sb[:, k * 128: (k + 1) * 128],
                            start=firstmm,
                            stop=(m_ == kf2 - 1 and k == kf1 - 1),
                        )
                        firstmm = False
                h1T_sb = wpool.tile([128, f2], cdt, tag="h1T")
                nc.scalar.activation(
                    h1T_sb[:, :], ps_c1[:, :], mybir.ActivationFunctionType.Relu
                )
                ps_h2 = ps_h2_p.tile([128, f3], f32, tag="h2")
                for k in range(kf2):
                    nc.tensor.matmul(
                        ps_h2[:, :],
                        lhsT=h1T_sb[:, k * 128: (k + 1) * 128],
                        rhs=w2_sb[:, k * f3: (k + 1) * f3],
                        start=(k == 0),
                        stop=(k == kf2 - 1),
                    )
                h2_sb = wpool.tile([128, f3], cdt, tag="h2sb")
                nc.scalar.copy(h2_sb[:, :], ps_h2[:, :])
                g = grp_of_block[b]
                off = (b - grp_b0[g]) * 128
                nc.sync.dma_start(
                    out=cc_in_g[g][off: off + nb, :], in_=h2_sb[:nb, :]
                )

                # interleave layer-2 group 0 in small chunks once its
                # AllGather has certainly completed, so TensorE fills its
                # layer-1 idle time without delaying the later blocks'
                # transforms (which gate the remaining AllGathers)
                if b == nblk - 4:
                    emit_h2f_load(0)
                    emit_l2(0, range(0, 3))
                elif b == nblk - 3:
                    emit_l2(0, range(3, 6))
                elif b == nblk - 2:
                    emit_l2(0, range(6, 8))
                elif b == nblk - 1:
                    emit_h2f_load(1)
                    emit_l2(0, range(8, nblk))

            # ---------------- remaining collectives + layer 2 ---------------
            for g in range(ngrp):
                if grp_end[g] == nblk - 1:
                    emit_cc(g)
            for g in range(1, ngrp):
                if h2f_g[g] is None:
                    emit_h2f_load(g)
                emit_l2(g, range(nblk))

    nc.finalize()
    return nc


def _make_in_maps(x, W1, W2, g, n_cores, npc, nblk):
    maps = []
    for c in range(n_cores):
        xloc = np.zeros((nblk * 128, x.shape[1]), dtype=x.dtype)
        xloc[:npc] = x[c * npc: (c + 1) * npc]
        maps.append({
            "x": x,
            "xloc": np.ascontiguousarray(xloc),
            "w1": W1,
            "w2": W2,
            "ind": g["ind"][c],
            "idxw": g["idxw"][c],
            "dloc": g["dloc"][c],
            "ind2": g["ind2"][c],
            "cnts": g["cnts"][c],
        })
    return maps


def build_all(x, edge_index, edge_weight, W1, W2, n_cores=N_CORES,
              compute_dtype=COMPUTE_DTYPE):
    """Host packing + Bass graph for the given full inputs."""
    if compute_dtype == "bf16":
        import ml_dtypes

        np_cdt = ml_dtypes.bfloat16
    else:
        np_cdt = np.float32
    x = np.ascontiguousarray(np.asarray(x, dtype=np.float32).astype(np_cdt))
    W1 = np.ascontiguousarray(np.asarray(W1, dtype=np.float32).astype(np_cdt))
    W2 = np.ascontiguousarray(np.asarray(W2, dtype=np.float32).astype(np_cdt))
    n_nodes, f1 = x.shape
    f2, f3 = W1.shape[1], W2.shape[1]
    g = _pack_graph(edge_index, edge_weight, n_nodes, n_cores,
                    ind_np_dtype=np_cdt, split_blocks=SPLIT_BLOCKS)
    nc = _build_nc(
        n_nodes, f1, f2, f3, g["npc"], g["nblk"], g["t_blocks"], g["tile_off"],
        n_cores, compute_dtype=compute_dtype, split_blocks=g["split_blocks"],
        grp_rows=g["grp_rows"], grp_pos0=g["grp_pos0"], calls=g["calls"],
        n_calls=g["n_calls"], ns_tiles=g["ns_tiles"],
    )
    return nc, _make_in_maps(x, W1, W2, g, n_cores, g["npc"], g["nblk"]), g


def kernel(x, edge_index, edge_weight, W1, W2):
    from concourse.bass_utils import run_bass_kernel_spmd

    nc, in_maps, _ = build_all(x, edge_index, edge_weight, W1, W2)
    res = run_bass_kernel_spmd(nc, in_maps, list(range(N_CORES)))
    out = np.concatenate(
        [np.asarray(res.results[c]["out"]) for c in range(N_CORES)], axis=0
    )
    return out.astype(np.float32)

